# revision 16
# baseline (speedup 1.0000x reference)
"""TRN2 Bass kernel: batch-invariant full attention.

Problem: out = softmax(Q K^T / sqrt(64)) V with Q,K,V f32 [4, 16, 2048, 64].
Sharding: the 64 (batch, head) pairs are split 8 ways across the 8
NeuronCores (8 pairs per core); attention is independent per pair.

Per-core design (per pair), v4 — engine-balanced pipeline:
  - Sequence indices are permuted as s = p*T + t (T=16) so every DMA is
    contiguous per partition; consistent between K and V and undone by
    construction on the output path.
  - Inputs are cast f32->fp16 during the load DMA (SWDGE cast); the
    1/sqrt(d) scale is folded into the exp. The next pair's loads +
    transposes are issued before the current pair's main loop (prefetch)
    so the PE never waits on the frontend.
  - Q^T/K^T are built by ONE xbar-transpose DMA each ([128,1024] ->
    [128, 8, 128]): logical row t*64+d lands on partition d+64*(t%2),
    slot t//2, i.e. the row-pair interleaved Q^T/K^T layout directly.
    A partition-swapped copy (kt2s) covers the parity cross terms.
  - QK runs as concurrent row-group pairs (tile_position from
    base_partition 0/64): per (m, cross) step, 4 N=512 matmuls produce
    S^T[k-tile, even-q 1024] and S^T[k-tile', odd-q 1024] f32 in two
    2-bank PSUM tiles.
  - exp splits across engines per tile of [128,1024]: ScalarE exact
    e^(sc*x), DVE one-instruction base-2 Schraudolph bitcast
    (i16 = x*(1024*sc*log2 e) + KOFF viewed as fp16, ~±3.5% on those
    weights; the softmax denominator uses the same approximated values
    so the error largely cancels). Assignment is per-lane uniform so
    every q column gets the same exact/approx mix over k.
  - PV is FLIPPED and lagged one step: stationary = [V | 1] fp16
    (65 cols per k-tile), moving = e (N=512), accumulating
    out^T = [O; denom] [65, 4, 512] in 4 PSUM banks. The one-step lag
    keeps the PE fed while exp of the current step is still running.
  - Epilogue: 4 strided DVE copies assemble out^T [65, 2048] fp16 with
    column order t*128+j; one xbar transposes it back so q lands at
    [p=q//16, t=q%16] (the denom row rides along via 80-partition pad);
    GpSimd reciprocal + per-t multiplies produce f32; contiguous DMA out.
"""
import functools
from contextlib import ExitStack

import numpy as np

import concourse.mybir as mybir
import concourse.tile as tile
from concourse import bacc
from concourse.bass_utils import run_bass_kernel_spmd

F32 = mybir.dt.float32
F16 = mybir.dt.float16
I16 = mybir.dt.int16
EXP = mybir.ActivationFunctionType.Exp
MULT = mybir.AluOpType.mult
ADD = mybir.AluOpType.add

B, H, S, D = 4, 16, 2048, 64
N_CORES = 8
NBH = B * H // N_CORES  # 8 (b,h) pairs per core

SC = 1.0 / 8.0  # 1/sqrt(D)
# DVE base-2 bitcast exp: i16 = trunc(z*1024*sc*log2(e) + KOFF), viewed as
# fp16. KOFF = 15*1024 (bias) - 36.2 (minimax ratio offset) + 0.5
# (truncation compensation).
KSC = float(1024.0 * SC * np.log2(np.e))
KOFF = float(15 * 1024 - 36.2 + 0.5)


def build_attention(nbh=NBH, S=S, D=D):
    assert D == 64
    T = S // 128  # 16 s-tiles of 128
    M = T // 2  # 8 tile pairs

    nc = bacc.Bacc("TRN2", target_bir_lowering=False, debug=False)
    q = nc.dram_tensor("q", [nbh, S, D], F32, kind="ExternalInput").ap()
    k = nc.dram_tensor("k", [nbh, S, D], F32, kind="ExternalInput").ap()
    v = nc.dram_tensor("v", [nbh, S, D], F32, kind="ExternalInput").ap()
    o = nc.dram_tensor("o", [nbh, S, D], F32, kind="ExternalOutput").ap()

    with tile.TileContext(nc) as tc, ExitStack() as ctx:
        ld = ctx.enter_context(tc.tile_pool(name="ld", bufs=2))
        tp = ctx.enter_context(tc.tile_pool(name="tp", bufs=2))
        ep = ctx.enter_context(tc.tile_pool(name="ep", bufs=6))
        ot = ctx.enter_context(tc.tile_pool(name="ot", bufs=2))
        of = ctx.enter_context(tc.tile_pool(name="of", bufs=2))
        pp_s = ctx.enter_context(tc.tile_pool(name="pp_s", bufs=2, space="PSUM"))
        pp_o = ctx.enter_context(tc.tile_pool(name="pp_o", bufs=1, space="PSUM"))

        def frontend(bh):
            """Loads + transposes for pair bh; returns operand tiles."""
            q16 = ld.tile([128, T, D], F16, tag="q16", name=f"q16_{bh}")
            k16 = ld.tile([128, T, D], F16, tag="k16", name=f"k16_{bh}")
            vaug = ld.tile([128, T, D + 1], F16, tag="vaug", name=f"vaug_{bh}")
            nc.gpsimd.dma_start(
                out=q16, in_=q[bh].rearrange("(p t) d -> p t d", p=128)
            )
            nc.gpsimd.dma_start(
                out=k16, in_=k[bh].rearrange("(p t) d -> p t d", p=128)
            )
            nc.gpsimd.dma_start(
                out=vaug[:, :, 0:D], in_=v[bh].rearrange("(p t) d -> p t d", p=128)
            )
            nc.gpsimd.memset(vaug[:, :, D : D + 1], 1.0)
            # one xbar per tensor: qt2[0:64, m, j] = Q^T[d, tile 2m, col j],
            # qt2[64:128, m, j] = Q^T[d, tile 2m+1, col j]   (q = j*T + t)
            qt2 = tp.tile([128, M, 128], F16, tag="qt2", name=f"qt2_{bh}")
            kt2 = tp.tile([128, M, 128], F16, tag="kt2", name=f"kt2_{bh}")
            kt2s = tp.tile([128, M, 128], F16, tag="kt2s", name=f"kt2s_{bh}")
            nc.sync.dma_start(out=qt2, in_=q16, transpose=True)
            nc.sync.dma_start(out=kt2, in_=k16, transpose=True)
            nc.gpsimd.dma_start(out=kt2s[0:64], in_=kt2[64:128])
            nc.gpsimd.dma_start(out=kt2s[64:128], in_=kt2[0:64])
            return (
                vaug,
                qt2.rearrange("p m j -> p (m j)"),
                kt2.rearrange("p m j -> p (m j)"),
                kt2s.rearrange("p m j -> p (m j)"),
            )

        fe = frontend(0)
        for bh in range(nbh):
            vaug, qt2f, kt2f, kt2sf = fe

            # ---- QK -> exp -> PV (PV lagged one step) ----
            poT = pp_o.tile([65, 4, 512], F32, tag="poT", name=f"poT{bh}")
            pending = []
            for s in range(2 * M + 1):
                if s < 2 * M:
                    m, cross = s // 2, s % 2
                    kkf = kt2sf if cross else kt2f
                    tiles = []
                    for half in (0, 1):
                        kb = 2 * m + (cross if half == 0 else 1 - cross)
                        lo, hi = 64 * half, 64 * (half + 1)
                        ps = pp_s.tile(
                            [128, 1024], F32, tag="ps", name=f"ps{bh}_{s}_{half}"
                        )
                        # each matmul output must fit one 2KB PSUM bank
                        for c2 in (0, 1):
                            nc.tensor.matmul(
                                out=ps[:, 512 * c2 : 512 * (c2 + 1)],
                                lhsT=kkf[lo:hi, 128 * m : 128 * (m + 1)],
                                rhs=qt2f[lo:hi, 512 * c2 : 512 * (c2 + 1)],
                                start=True,
                                stop=True,
                            )
                        e = ep.tile([128, 1024], F16, tag="e")
                        # exp split BY COLUMN HALF across both engines so the
                        # ps tile drains in ~720ns (not 1147): ScalarE takes
                        # half c2==(s+half)%2 except on steps {7,14} (both
                        # halves to DVE, keeping ScalarE under the PE budget).
                        # Each q column sees a uniform 7-exact/9-approx mix
                        # over its 16 k-steps.
                        sc_half = (s + half) % 2
                        for c2 in (0, 1):
                            ec = e[:, 512 * c2 : 512 * (c2 + 1)]
                            pc = ps[:, 512 * c2 : 512 * (c2 + 1)]
                            if c2 == sc_half and s not in (7, 14):
                                nc.scalar.activation(
                                    out=ec, in_=pc, func=EXP, scale=SC
                                )
                            else:
                                nc.vector.tensor_scalar(
                                    out=ec.bitcast(I16),
                                    in0=pc,
                                    scalar1=KSC,
                                    scalar2=KOFF,
                                    op0=MULT,
                                    op1=ADD,
                                )
                        tiles.append((e, kb, half))
                    pending.append((s, tiles))
                    if s == 0 and bh + 1 < nbh:
                        # prefetch next pair's frontend early so its loads
                        # aren't queued behind this pair's epilogue
                        fe = frontend(bh + 1)
                if s >= 1:
                    sp, tiles = pending.pop(0)
                    for e, kb, half in tiles:
                        for c in (0, 1):
                            nc.tensor.matmul(
                                out=poT[:, 2 * half + c, :],
                                lhsT=vaug[:, kb, :],
                                rhs=e[:, 512 * c : 512 * (c + 1)],
                                start=sp == 0,
                                stop=sp == 2 * M - 1,
                            )

            # ---- epilogue ----
            # outT col C = t*128 + j holds q = j*16 + t, so the xbar (which
            # writes logical row r to partition r%128, free slot r//128)
            # lands q exactly at [p=q//16, t=q%16]. Padded to 80 partitions
            # (xbar needs %16) so the denom row rides the same transpose.
            outT = ot.tile([80, S], F16, tag="outT")
            outT_r = outT[0:65].rearrange("p (t j) -> p t j", t=16)
            for j4 in range(4):
                half, c = j4 // 2, j4 % 2
                base = 8 * c + half
                nc.vector.tensor_copy(
                    out=outT_r[:, base : base + 7 : 2, :],
                    in_=poT[:, j4, :].rearrange("p (mq j) -> p mq j", mq=4),
                )
            out16x = of.tile([128, T, 80], F16, tag="out16x")
            nc.sync.dma_start(out=out16x, in_=outT, transpose=True)
            rcp = of.tile([128, T], F32, tag="rcp")
            nc.vector.reciprocal(out=rcp, in_=out16x[:, :, 64])
            outf = of.tile([128, T, D], F32, tag="outf")
            nc.vector.tensor_tensor(
                out=outf,
                in0=out16x[:, :, 0:D],
                in1=rcp.broadcast_to((128, T, D)),
                op=MULT,
            )
            nc.gpsimd.dma_start(
                out=o[bh].rearrange("(p t) d -> p t d", p=128), in_=outf
            )
    nc.compile()
    return nc


@functools.lru_cache(maxsize=1)
def _built():
    return build_attention()


def run(query, key, value, trace=False):
    """Shard (b,h) pairs 8 ways, run on cores 0-7, gather. Returns
    (out [B,H,S,D] f32, BassKernelResults)."""
    nc = _built()
    qf = np.ascontiguousarray(np.asarray(query, dtype=np.float32).reshape(B * H, S, D))
    kf = np.ascontiguousarray(np.asarray(key, dtype=np.float32).reshape(B * H, S, D))
    vf = np.ascontiguousarray(np.asarray(value, dtype=np.float32).reshape(B * H, S, D))
    in_maps = []
    for c in range(N_CORES):
        sl = slice(c * NBH, (c + 1) * NBH)
        in_maps.append(
            {
                "q": np.ascontiguousarray(qf[sl]),
                "k": np.ascontiguousarray(kf[sl]),
                "v": np.ascontiguousarray(vf[sl]),
            }
        )
    res = None
    last_err = None
    for attempt in range(3):
        try:
            res = run_bass_kernel_spmd(
                nc, in_maps, core_ids=list(range(N_CORES)), trace=trace
            )
            break
        except Exception as e:  # transient device wedge: retry
            last_err = e
            import time as _time

            _time.sleep(5 * (attempt + 1))
    if res is None:
        raise last_err
    out = np.concatenate([res.results[c]["o"] for c in range(N_CORES)], axis=0)
    return out.reshape(B, H, S, D).astype(np.float32), res


def kernel(query, key, value):
    out, _ = run(query, key, value)
    return out


# revision 21
# speedup vs baseline: 1.6108x; 1.6108x over previous
"""TRN2 Bass kernel: batch-invariant full attention.

Problem: out = softmax(Q K^T / sqrt(64)) V with Q,K,V f32 [4, 16, 2048, 64].
Sharding: the 64 (batch, head) pairs are split 8 ways across the 8
NeuronCores (8 pairs per core); attention is independent per pair.

Per-core design (per pair):
  - Sequence indices are permuted as s = p*T + t so every DMA is
    contiguous per partition; the permutation is consistent between K and
    V (softmax invariant) and undone on the output write.
  - All matmul operands are fp16 (11-bit mantissa ~ tf32-grade rounding,
    2-byte so the PE streams 1 row/cycle; fp32 accumulation in PSUM).
  - Q^T / K^T are built with fp16 PE pair-transposes [128,128] -> fp16
    PSUM, giving a row-pair interleaved layout: even tiles on partitions
    0-63, odd on 64-127. QK matmuls run as concurrent row-group pairs;
    a partition-swapped DMA copy (kt2s) covers the parity cross terms.
  - Scores come out transposed, S^T[k, q]; exp on ScalarE (PSUM->SBUF,
    scale folded into the activation affine, fp16 out).
  - PV uses the exp tile as the *stationary* operand (fp16 128-col
    weights -> fast weight load) and V (augmented with a ones column)
    as the 65-row moving operand, accumulating [O ; denom] directly in
    q-major layout [128, 65] per q block. No output transposes.
  - Epilogue: reciprocal of the denom column + tensor_scalar multiply
    straight out of PSUM, contiguous DMA out.
"""
import functools
from contextlib import ExitStack

import numpy as np

import concourse.mybir as mybir
import concourse.tile as tile
from concourse import bacc
from concourse.bass_utils import run_bass_kernel_spmd
from concourse.masks import make_identity

F32 = mybir.dt.float32
F16 = mybir.dt.float16
I16 = mybir.dt.int16
EXP = mybir.ActivationFunctionType.Exp
MULT = mybir.AluOpType.mult
ADD = mybir.AluOpType.add
# DVE base-2 bitcast exp (scores are log2-scaled): i16 = trunc(z*1024 +
# KOFF) viewed as fp16; KOFF = 15*1024 - 36.2 (minimax offset) + 0.5
# (truncation compensation). ~±3.5% on those weights; the softmax
# denominator uses the same approximated values so the bias cancels.
KOFF = float(15 * 1024 - 36.2 + 0.5)

B, H, S, D = 4, 16, 2048, 64
N_CORES = 8
NBH = B * H // N_CORES  # 8 (b,h) pairs per core


def build_attention(nbh=NBH, S=S, D=D):
    assert D == 64
    T = S // 128  # 16 k/q tiles of 128
    M = T // 2  # 8 tile pairs
    QCN = 2  # q chunks (1024 each for S=2048)
    qhalf = S // QCN // 2  # 512: even-parity half of a q chunk
    assert qhalf % 512 == 0  # row-pair outputs must land in distinct PSUM banks
    nblk = qhalf // 128  # 4 q blocks per parity half
    scale = 1.0 / float(np.sqrt(D))

    nc = bacc.Bacc("TRN2", target_bir_lowering=False, debug=False)
    q = nc.dram_tensor("q", [nbh, S, D], F32, kind="ExternalInput").ap()
    k = nc.dram_tensor("k", [nbh, S, D], F32, kind="ExternalInput").ap()
    v = nc.dram_tensor("v", [nbh, S, D], F32, kind="ExternalInput").ap()
    o = nc.dram_tensor("o", [nbh, S, D], F32, kind="ExternalOutput").ap()

    with tile.TileContext(nc) as tc, ExitStack() as ctx:
        singles = ctx.enter_context(tc.tile_pool(name="singles", bufs=1))
        ident = singles.tile([128, 128], F16)
        make_identity(nc, ident)

        ld = ctx.enter_context(tc.tile_pool(name="ld", bufs=2))
        c16 = ctx.enter_context(tc.tile_pool(name="c16", bufs=2))
        persist = ctx.enter_context(tc.tile_pool(name="persist", bufs=2))
        epool = ctx.enter_context(tc.tile_pool(name="epool", bufs=4))
        opool = ctx.enter_context(tc.tile_pool(name="opool", bufs=2))
        pp_s = ctx.enter_context(tc.tile_pool(name="pp_s", bufs=2, space="PSUM"))
        pp_t = ctx.enter_context(tc.tile_pool(name="pp_t", bufs=2, space="PSUM"))
        pp_o = ctx.enter_context(tc.tile_pool(name="pp_o", bufs=1, space="PSUM"))

        for bh in range(nbh):
            # ---- load (s = p*T + t permutation; K/V cast f32->fp16 during
            # the DMA itself -- SWDGE cast) ----
            qn = ld.tile([128, T, D], F32, tag="qn")
            kn16 = c16.tile([128, T, D], F16, tag="kn16")
            vaug = persist.tile([128, T, D + 1], F16, tag="vaug")
            qv = q[bh].rearrange("(p t) d -> p t d", p=128)
            kv = k[bh].rearrange("(p t) d -> p t d", p=128)
            if bh == 0:
                H2 = T // 4
                nc.gpsimd.dma_start(out=qn[:, 0:H2, :], in_=qv[:, 0:H2, :])
                nc.gpsimd.dma_start(out=kn16[:, 0:H2, :], in_=kv[:, 0:H2, :])
                nc.gpsimd.dma_start(out=qn[:, H2:T, :], in_=qv[:, H2:T, :])
                nc.gpsimd.dma_start(out=kn16[:, H2:T, :], in_=kv[:, H2:T, :])
            else:
                nc.gpsimd.dma_start(out=qn, in_=qv)
                nc.gpsimd.dma_start(out=kn16, in_=kv)
            nc.gpsimd.dma_start(
                out=vaug[:, :, 0:D], in_=v[bh].rearrange("(p t) d -> p t d", p=128)
            )
            nc.gpsimd.memset(vaug[:, :, D : D + 1], 1.0)

            # ---- Q fp16 scale-cast (DVE) ----
            qn16 = c16.tile([128, T, D], F16, tag="qn16")
            # scale*log2(e) folded into the Q cast: scores become log2-scaled,
            # so exp is computed as 2^z = e^(z*ln2) (DVE fast-exp ready).
            sc = float(scale * np.log2(np.e))
            if bh == 0:
                nc.vector.tensor_scalar_mul(out=qn16[:, 0:H2, :], in0=qn[:, 0:H2, :], scalar1=sc)
                nc.vector.tensor_scalar_mul(out=qn16[:, H2:T, :], in0=qn[:, H2:T, :], scalar1=sc)
            else:
                nc.vector.tensor_scalar_mul(out=qn16, in0=qn, scalar1=sc)

            # ---- PE pair-transposes: qt2/kt2 [128, M, 128] interleaved ----
            # qt2[0:64, m, j] = Q^T[d, q tile 2m, col j] (tile col j <-> s = j*T + 2m)
            # qt2[64:128, m, j] = Q^T[d, q tile 2m+1, col j]
            qt2 = persist.tile([128, M, 128], F16, tag="qt2")
            kt2 = persist.tile([128, M, 128], F16, tag="kt2")
            kt2s = persist.tile([128, M, 128], F16, tag="kt2s")
            for m in range(M):
                ptq = pp_t.tile([128, 128], F16, tag="ptr", name=f"ptq{bh}_{m}")
                nc.tensor.transpose(
                    out=ptq, in_=qn16[:, 2 * m : 2 * m + 2, :], identity=ident
                )
                nc.vector.tensor_copy(out=qt2[:, m, :], in_=ptq)
                ptk = pp_t.tile([128, 128], F16, tag="ptr", name=f"ptk{bh}_{m}")
                nc.tensor.transpose(
                    out=ptk, in_=kn16[:, 2 * m : 2 * m + 2, :], identity=ident
                )
                nc.vector.tensor_copy(out=kt2[:, m, :], in_=ptk)
                # per-pair partition-swapped copy for the parity cross terms,
                # so QK iteration m only depends on its own tiles
                nc.gpsimd.dma_start(out=kt2s[0:64, m, :], in_=kt2[64:128, m, :])
                nc.gpsimd.dma_start(out=kt2s[64:128, m, :], in_=kt2[0:64, m, :])

            qt2f = qt2.rearrange("p m j -> p (m j)")
            kt2f = kt2.rearrange("p m j -> p (m j)")
            kt2sf = kt2s.rearrange("p m j -> p (m j)")

            # ---- QK -> exp -> PV ----
            for qc in range(QCN):
                # poq[:, c, :] accumulates [O ; denom] for q block c of this
                # chunk: c < nblk are even-parity q tiles, c >= nblk odd.
                # padded to 128 f32 per block so each [128, 65] matmul output
                # stays inside one 2KB PSUM bank
                poq = pp_o.tile([128, 2 * nblk, 128], F32, tag="poq")
                rhs_lo = qt2f[0:64, qc * qhalf : (qc + 1) * qhalf]
                rhs_hi = qt2f[64:128, qc * qhalf : (qc + 1) * qhalf]
                for m in range(M):
                    for cross in (0, 1):
                        kk = kt2sf if cross else kt2f
                        kb_lo = 2 * m + cross
                        kb_hi = 2 * m + 1 - cross
                        ps = pp_s.tile([128, 2 * qhalf], F32, tag="ps")
                        nc.tensor.matmul(
                            out=ps[:, 0:qhalf],
                            lhsT=kk[0:64, 128 * m : 128 * (m + 1)],
                            rhs=rhs_lo,
                            start=True,
                            stop=True,
                        )
                        nc.tensor.matmul(
                            out=ps[:, qhalf : 2 * qhalf],
                            lhsT=kk[64:128, 128 * m : 128 * (m + 1)],
                            rhs=rhs_hi,
                            start=True,
                            stop=True,
                        )
                        e = epool.tile([128, 2 * qhalf], F16, tag="e")
                        # exp split across engines: ScalarE exact 2^z on half
                        # the tiles, DVE one-instruction bitcast exp2 on the
                        # rest. (m+cross) parity alternates so every q column
                        # sees a uniform 8-exact/8-approx mix over its k tiles.
                        if (qc + m + cross) % 2 == 0:
                            nc.scalar.activation(
                                out=e, in_=ps, func=EXP, scale=float(np.log(2.0))
                            )
                        else:
                            nc.vector.tensor_scalar(
                                out=e.bitcast(I16),
                                in0=ps,
                                scalar1=1024.0,
                                scalar2=KOFF,
                                op0=MULT,
                                op1=ADD,
                            )
                        first = m == 0 and cross == 0
                        last = m == M - 1 and cross == 1
                        for c in range(2 * nblk):
                            kb = kb_lo if c < nblk else kb_hi
                            # start=True clears the ENTIRE psum bank, so only
                            # the first matmul touching each bank may set it;
                            # per-element has_written handles the other blocks.
                            nc.tensor.matmul(
                                out=poq[:, c, 0 : D + 1],
                                lhsT=e[:, 128 * c : 128 * (c + 1)],
                                rhs=vaug[:, kb, :],
                                start=first and c % nblk == 0,
                                stop=last,
                            )

                # ---- epilogue: one copy frees poq's PSUM banks early, then
                # normalize from SBUF ----
                ocp = opool.tile([128, 2 * nblk, D + 1], F32, tag="ocp")
                nc.vector.tensor_copy(out=ocp, in_=poq[:, :, 0 : D + 1])
                outsb = opool.tile([128, 2 * nblk, D], F32, tag="outsb")
                # batched reciprocal of all 8 denom columns, then one
                # broadcast multiply; c = par*nblk + blk maps to
                # tt_local = 2*blk + par via the rearranged output AP.
                rcp = opool.tile([128, 2 * nblk], F32, tag="rcp")
                nc.vector.reciprocal(out=rcp, in_=ocp[:, :, D])
                nc.vector.tensor_tensor(
                    out=outsb.rearrange("p (blk par) d -> p par blk d", par=2),
                    in0=ocp[:, :, 0:D].rearrange("p (par blk) d -> p par blk d", par=2),
                    in1=rcp.rearrange("p (par blk) -> p par blk", par=2).broadcast_to(
                        (128, 2, nblk, D)
                    ),
                    op=MULT,
                )
                nc.gpsimd.dma_start(
                    out=o[bh].rearrange("(p t) d -> p t d", p=128)[
                        :, qc * 2 * nblk : (qc + 1) * 2 * nblk, :
                    ],
                    in_=outsb,
                )
    nc.compile()
    return nc


@functools.lru_cache(maxsize=1)
def _built():
    return build_attention()


def run(query, key, value, trace=False):
    """Shard (b,h) pairs 8 ways, run on cores 0-7, gather. Returns
    (out [B,H,S,D] f32, BassKernelResults)."""
    nc = _built()
    qf = np.ascontiguousarray(np.asarray(query, dtype=np.float32).reshape(B * H, S, D))
    kf = np.ascontiguousarray(np.asarray(key, dtype=np.float32).reshape(B * H, S, D))
    vf = np.ascontiguousarray(np.asarray(value, dtype=np.float32).reshape(B * H, S, D))
    in_maps = []
    for c in range(N_CORES):
        sl = slice(c * NBH, (c + 1) * NBH)
        in_maps.append(
            {
                "q": np.ascontiguousarray(qf[sl]),
                "k": np.ascontiguousarray(kf[sl]),
                "v": np.ascontiguousarray(vf[sl]),
            }
        )
    res = None
    last_err = None
    for attempt in range(3):
        try:
            res = run_bass_kernel_spmd(
                nc, in_maps, core_ids=list(range(N_CORES)), trace=trace
            )
            break
        except Exception as e:  # transient device wedge: retry
            last_err = e
            import time as _time

            _time.sleep(5 * (attempt + 1))
    if res is None:
        raise last_err
    out = np.concatenate([res.results[c]["o"] for c in range(N_CORES)], axis=0)
    return out.reshape(B, H, S, D).astype(np.float32), res


def kernel(query, key, value):
    out, _ = run(query, key, value)
    return out

